# revision 7
# baseline (speedup 1.0000x reference)
"""Trainium2 Bass kernel for dynamic-conv1d attention-scale module.

Computes out = x + x * scale where
  scale[b,c,h,w] = sum_k attn[b,k,h,w] * w_sum[k,c]
  attn = softmax_k(logits/T),  logits[b,k,h,w] = fc2 @ relu(fc1 * qm)
  w_sum = weight.sum(axis=1)

Device strategy (8 NeuronCores, data-parallel over batch x H-halves):
  * quality_map >= 0 and fc1 is a bias-free 1x1 conv =>
    relu(fc1_w * q) == q * relu(fc1_w), so logits[k] = g[k]*q + b2[k]
    with g = fc2_w @ relu(fc1_w) (host-side weight-only folding).
  * softmax rows sum to 1 => 1 + scale = sum_k attn_k * (w_sum[k,c] + 1),
    so one tiny PE matmul per tile produces (1+scale) in PSUM.
  * x / y stream in float16 (harness gate is rel_err < 2e-2; fp16 I/O
    keeps the end-to-end error ~4e-4 while halving HBM traffic, which is
    the roofline for this kernel).
  * Attention runs in a [72, 256] pixel layout; the pixel-major row
    matrix the matmul needs lives in SBUF as [9 chunks x 4 k] = 36
    partitions (chunk-major), produced by 4 SBUF->SBUF flatten DMAs.
    36 dest partitions spread the writes over many AXI ports, so the
    flatten completes in ~1-2 us even while the x stream is running
    (a [4, N] layout funnels through one port and takes >10 us).
  * The output multiply ot = x * (1+scale) is split between the DVE
    (1322 cols) and GpSimd (726 cols) so per-tile production (~1.5 us)
    stays under the DMA fabric time (~2.35 us/tile); a single DVE
    tensor_tensor (PSUM operand => 1x mode) would cap the pipeline at
    ~218 GB/s.
Each core streams its 9.4 MB x-shard in [128 x 2048] fp16 tiles,
writing 9.4 MB back, ~19 MB total HBM traffic per core.
"""

import sys

if "/opt/trn_rl_repo" not in sys.path:
    sys.path.insert(0, "/opt/trn_rl_repo")

import numpy as np

import concourse.bacc as bacc
import concourse.mybir as mybir
from concourse.bass_utils import run_bass_kernel_spmd
from concourse.tile import TileContext

_B, _C, _H, _W = 4, 256, 192, 192
_K = 4
_TEMP = 34.0
_NCORES = 8
_HS = _H // 2            # 96 rows of H per shard
_N = _HS * _W            # 18432 pixels per core
_P = 128                 # SBUF partitions
_AP_ = 72                # attention partitions
_AF = _N // _AP_         # 256 pixels per attention partition
_CH = 2048               # pixels per main-loop tile (4 KB/partition fp16)
_NT = _N // _CH          # 9 chunks
_PG = _CH // _AF         # 8 attention partitions per chunk
_MM = 512                # matmul moving free dim (one PSUM bank)
_VS = 1344               # DVE's share of the output-multiply columns
_F32 = mybir.dt.float32
_F16 = mybir.dt.float16


def _build_nc():
    nc = bacc.Bacc()
    x_d = nc.dram_tensor("x", [_C, _N], _F16, kind="ExternalInput")
    qm_d = nc.dram_tensor("qm", [_AP_, _AF], _F32, kind="ExternalInput")
    w_d = nc.dram_tensor("w", [_K, _C], _F16, kind="ExternalInput")
    g_d = nc.dram_tensor("g", [_AP_, 2 * _K], _F32, kind="ExternalInput")
    y_d = nc.dram_tensor("y", [_C, _N], _F16, kind="ExternalOutput")

    with TileContext(nc) as tc:
        with (
            tc.tile_pool(name="const", bufs=1) as cpool,
            tc.tile_pool(name="attn", bufs=1) as apool,
            tc.tile_pool(name="xin", bufs=8) as xpool,
            tc.tile_pool(name="rstage", bufs=4) as spool,
            tc.tile_pool(name="yout", bufs=4) as ypool,
            tc.tile_pool(name="ps", bufs=2, space="PSUM") as pspool,
        ):
            # Small loads ride the scalar queue so the sync queue is a pure
            # x stream; all of them land before the attention math needs them.
            gt = cpool.tile([_AP_, 2 * _K], _F32)
            q = apool.tile([_AP_, _AF], _F32)
            wt = cpool.tile([_K, _C], _F16)
            nc.scalar.dma_start(out=gt[:, :], in_=g_d[:, :])
            nc.scalar.dma_start(out=q[:, :], in_=qm_d[:, :])
            nc.scalar.dma_start(out=wt[:, :], in_=w_d[:, :])

            # ---- attention pointwise in [72, 256] layout ----
            e = apool.tile([_AP_, _K * _AF], _F32)
            for k in range(_K):
                # e_k = exp((g_k/T) * q + b_k/T)
                nc.scalar.activation(
                    out=e[:, k * _AF : (k + 1) * _AF],
                    in_=q[:, :],
                    func=mybir.ActivationFunctionType.Exp,
                    bias=gt[:, _K + k : _K + k + 1],
                    scale=gt[:, k : k + 1],
                )
            d0 = apool.tile([_AP_, _AF], _F32)
            d1 = apool.tile([_AP_, _AF], _F32)
            nc.vector.tensor_add(
                out=d0[:, :], in0=e[:, 0:_AF], in1=e[:, _AF : 2 * _AF]
            )
            nc.vector.tensor_add(
                out=d1[:, :], in0=e[:, 2 * _AF : 3 * _AF], in1=e[:, 3 * _AF :]
            )
            nc.vector.tensor_add(out=d0[:, :], in0=d0[:, :], in1=d1[:, :])
            r = apool.tile([_AP_, _AF], _F32)
            nc.vector.reciprocal_approx_accurate(
                out=r[:, :], in_=d0[:, :], scratch=d1[:, :]
            )
            # attn in fp16 (k-blocked per partition), then flatten each k
            # into the chunk-major rows layout: partition g*4+k holds the
            # 2048 attn_k values of chunk g.
            a16 = apool.tile([_AP_, _K * _AF], _F16)
            rows = apool.tile([_NT * _K, _CH], _F16)
            for k in range(_K):
                nc.vector.tensor_mul(
                    out=a16[:, k * _AF : (k + 1) * _AF],
                    in0=e[:, k * _AF : (k + 1) * _AF],
                    in1=r[:, :],
                )
                nc.scalar.dma_start(
                    out=rows[k : _NT * _K : _K, :].rearrange(
                        "g (p f) -> g p f", p=_PG
                    ),
                    in_=a16[:, k * _AF : (k + 1) * _AF],
                )

            # ---- main stream: out = x * (1 + scale) ----
            for t in range(_NT):
                nsl = slice(t * _CH, (t + 1) * _CH)
                # PE operands must sit at a 32-aligned base partition, so
                # restage this chunk's 4 rows from the lattice to base 0.
                rt = spool.tile([_K, _CH], _F16)
                nc.sync.dma_start(
                    out=rt[:, :], in_=rows[t * _K : (t + 1) * _K, :]
                )
                for ch in range(_C // _P):
                    lhsT = wt[:, ch * _P : (ch + 1) * _P]
                    xt = xpool.tile([_P, _CH], _F16)
                    nc.sync.dma_start(
                        out=xt[:, :], in_=x_d[ch * _P : (ch + 1) * _P, nsl]
                    )
                    ps = pspool.tile([_P, _CH], _F32)
                    for j in range(_CH // _MM):
                        nc.tensor.matmul(
                            ps[:, j * _MM : (j + 1) * _MM],
                            lhsT,
                            rt[:, j * _MM : (j + 1) * _MM],
                            start=True,
                            stop=True,
                        )
                    ot = ypool.tile([_P, _CH], _F16)
                    nc.vector.tensor_mul(
                        out=ot[:, 0:_VS], in0=xt[:, 0:_VS], in1=ps[:, 0:_VS]
                    )
                    # GPSIMD cannot read PSUM: scalar engine stages the tail
                    # columns to SBUF (fp16), gpsimd multiplies from there.
                    cs = ypool.tile([_P, _CH - _VS], _F16)
                    nc.scalar.copy(out=cs[:, :], in_=ps[:, _VS:])
                    nc.gpsimd.tensor_mul(
                        out=ot[:, _VS:], in0=xt[:, _VS:], in1=cs[:, :]
                    )
                    nc.scalar.dma_start(
                        out=y_d[ch * _P : (ch + 1) * _P, nsl], in_=ot[:, :]
                    )
    nc.compile()
    return nc


def _prepare_in_maps(x, quality_map, fc1_w, fc2_w, fc2_b, weight):
    x = np.asarray(x, dtype=np.float32)
    qm = np.asarray(quality_map, dtype=np.float32)
    fc1 = np.asarray(fc1_w, dtype=np.float32)
    fc2 = np.asarray(fc2_w, dtype=np.float32)
    b2 = np.asarray(fc2_b, dtype=np.float32)
    w = np.asarray(weight, dtype=np.float32)

    # Weight-only folding (host): g = fc2 @ relu(fc1); w1 = w_sum + 1.
    g = (fc2 @ np.maximum(fc1[:, 0], 0.0)).astype(np.float32)        # [K]
    w1 = (w.sum(axis=1) + 1.0).astype(np.float16)                    # [K, C]
    gb = np.concatenate([g / _TEMP, b2 / _TEMP]).astype(np.float32)  # [2K]
    gb_rep = np.ascontiguousarray(np.broadcast_to(gb, (_AP_, 2 * _K)))

    x16 = x.astype(np.float16)
    in_maps = []
    for core in range(_NCORES):
        b, half = divmod(core, 2)
        h0 = half * _HS
        xs = np.ascontiguousarray(x16[b, :, h0 : h0 + _HS, :]).reshape(_C, _N)
        qs = np.ascontiguousarray(qm[b, 0, h0 : h0 + _HS, :]).reshape(_AP_, _AF)
        in_maps.append({"x": xs, "qm": qs, "w": w1, "g": gb_rep})
    return in_maps


def _run(in_maps, **kwargs):
    nc = _build_nc()
    return run_bass_kernel_spmd(nc, in_maps, core_ids=list(range(_NCORES)), **kwargs)


def kernel(x, quality_map, fc1_w, fc2_w, fc2_b, weight):
    in_maps = _prepare_in_maps(x, quality_map, fc1_w, fc2_w, fc2_b, weight)
    res = _run(in_maps)
    out = np.empty((_B, _C, _H, _W), dtype=np.float32)
    for core in range(_NCORES):
        b, half = divmod(core, 2)
        h0 = half * _HS
        out[b, :, h0 : h0 + _HS, :] = (
            res.results[core]["y"].astype(np.float32).reshape(_C, _HS, _W)
        )
    return out


# revision 17
# speedup vs baseline: 1.0719x; 1.0719x over previous
"""Trainium2 Bass kernel for dynamic-conv1d attention-scale module.

Computes out = x + x * scale where
  scale[b,c,h,w] = sum_k attn[b,k,h,w] * w_sum[k,c]
  attn = softmax_k(logits/T),  logits[b,k,h,w] = fc2 @ relu(fc1 * qm)
  w_sum = weight.sum(axis=1)

Device strategy (8 NeuronCores, data-parallel over batch x H-halves):
  * quality_map >= 0 and fc1 is a bias-free 1x1 conv =>
    relu(fc1_w * q) == q * relu(fc1_w), so logits[k] = g[k]*q + b2[k]
    with g = fc2_w @ relu(fc1_w) (host-side weight-only folding).
  * softmax rows sum to 1 => 1 + scale = sum_k attn_k * (w_sum[k,c] + 1),
    so one tiny PE matmul per tile produces (1+scale) in PSUM.
  * x / y stream in float16 (harness gate is rel_err < 2e-2; fp16 I/O
    keeps the end-to-end error ~4e-4 while halving HBM traffic, which is
    the roofline for this kernel).
  * Attention runs in a [72, 256] pixel layout. A 4-DMA SBUF->SBUF
    flatten builds a chunk-major row lattice [9 chunks x 4 k, 2048]
    whose 36 dest partitions spread the writes over ~9 AXI ports (a
    [4, N] layout funnels through one 27 GB/s port).  PE operands must
    sit at base partition 0/32/64/96, so chunk 0 reads the lattice rows
    0-3 directly, chunk 8 reads rows 32-35 (w replicated at base 32 on
    host), and chunks 1-7 are restaged to base-0 tiles by small DMAs
    that run well ahead of their matmuls.  All small DMAs ride the
    scalar HWDGE ring AHEAD of the y writes (a small DMA stuck behind
    the x-tile FIFO, or racing the x stream at the SDMA engines, costs
    ~5 us per hop, so the critical path is kept to
    qm load -> pointwise -> flatten -> chunk-0 matmul).
  * The output multiply ot = x * (1+scale) is split three ways: DVE
    multiplies 1536 cols from PSUM (1x mode), the scalar engine stages
    the other 512 cols PSUM->SBUF (GPSIMD cannot read PSUM), gpsimd
    multiplies those. Per-tile production ~1.76 us stays under the DMA
    fabric time (~2.35 us/tile); a single DVE tensor_tensor would cap
    the write stream at ~218 GB/s.
Each core streams its 9.4 MB x-shard in [128 x 2048] fp16 tiles,
writing 9.4 MB back, ~19 MB total HBM traffic per core.
"""

import sys

if "/opt/trn_rl_repo" not in sys.path:
    sys.path.insert(0, "/opt/trn_rl_repo")

import numpy as np

import concourse.bacc as bacc
import concourse.mybir as mybir
from concourse.bass_utils import run_bass_kernel_spmd
from concourse.tile import TileContext

_B, _C, _H, _W = 4, 256, 192, 192
_K = 4
_TEMP = 34.0
_NCORES = 8
_HS = _H // 2            # 96 rows of H per shard
_N = _HS * _W            # 18432 pixels per core
_P = 128                 # SBUF partitions
_AP_ = 72                # attention partitions
_AF = _N // _AP_         # 256 pixels per attention partition
_CH = 2048               # pixels per main-loop tile (4 KB/partition fp16)
_NT = _N // _CH          # 9 chunks
_PG = _CH // _AF         # 8 attention partitions per chunk
_MM = 512                # matmul moving free dim (one PSUM bank)
_VS = 1536               # DVE's share of the output-multiply columns
_F32 = mybir.dt.float32
_F16 = mybir.dt.float16


def _build_nc():
    nc = bacc.Bacc()
    x_d = nc.dram_tensor("x", [_C, _N], _F16, kind="ExternalInput")
    qm_d = nc.dram_tensor("qm", [_AP_, _AF], _F32, kind="ExternalInput")
    w_d = nc.dram_tensor("w", [_P, _C], _F16, kind="ExternalInput")
    g_d = nc.dram_tensor("g", [_AP_, 2 * _K], _F32, kind="ExternalInput")
    y_d = nc.dram_tensor("y", [_C, _N], _F16, kind="ExternalOutput")

    with TileContext(nc) as tc:
        with (
            tc.tile_pool(name="const", bufs=1) as cpool,
            tc.tile_pool(name="attn", bufs=1) as apool,
            tc.tile_pool(name="rstage", bufs=_NT - 2) as spool,
            tc.tile_pool(name="xin", bufs=12) as xpool,
            tc.tile_pool(name="yout", bufs=4) as ypool,
            tc.tile_pool(name="cstage", bufs=4) as cpool2,
            tc.tile_pool(name="ps", bufs=2, space="PSUM") as pspool,
        ):
            # qm leads the scalar queue (the attention chain gates the write
            # stream); the sync queue is a pure x stream.
            gt = cpool.tile([_AP_, 2 * _K], _F32)
            q = apool.tile([_AP_, _AF], _F32)
            wt = cpool.tile([_P, _C], _F16)
            nc.scalar.dma_start(out=q[:, :], in_=qm_d[:, :])
            nc.scalar.dma_start(out=gt[:, :], in_=g_d[:, :])
            nc.scalar.dma_start(out=wt[:, :], in_=w_d[:, :])

            # ---- attention pointwise in [72, 256] layout ----
            e = apool.tile([_AP_, _K * _AF], _F32)
            for k in range(_K):
                # e_k = exp((g_k/T) * q + b_k/T)
                nc.scalar.activation(
                    out=e[:, k * _AF : (k + 1) * _AF],
                    in_=q[:, :],
                    func=mybir.ActivationFunctionType.Exp,
                    bias=gt[:, _K + k : _K + k + 1],
                    scale=gt[:, k : k + 1],
                )
            d0 = apool.tile([_AP_, _AF], _F32)
            d1 = apool.tile([_AP_, _AF], _F32)
            nc.vector.tensor_add(
                out=d0[:, :], in0=e[:, 0:_AF], in1=e[:, _AF : 2 * _AF]
            )
            nc.vector.tensor_add(
                out=d1[:, :], in0=e[:, 2 * _AF : 3 * _AF], in1=e[:, 3 * _AF :]
            )
            nc.vector.tensor_add(out=d0[:, :], in0=d0[:, :], in1=d1[:, :])
            r = apool.tile([_AP_, _AF], _F32)
            nc.vector.reciprocal_approx_accurate(
                out=r[:, :], in_=d0[:, :], scratch=d1[:, :]
            )
            # attn in fp16 (k-blocked per partition)
            a16 = apool.tile([_AP_, _K * _AF], _F16)
            for k in range(_K):
                nc.vector.tensor_mul(
                    out=a16[:, k * _AF : (k + 1) * _AF],
                    in0=e[:, k * _AF : (k + 1) * _AF],
                    in1=r[:, :],
                )
            # 4-DMA flatten into the chunk-major lattice:
            # lat[g*4+k, p*256+f] = a16[g*8+p, k*256+f]
            lat = apool.tile([_NT * _K, _CH], _F16)
            for k in range(_K):
                nc.scalar.dma_start(
                    out=lat[k : _NT * _K : _K, :].rearrange(
                        "g (p f) -> g p f", p=_PG
                    ),
                    in_=a16[:, k * _AF : (k + 1) * _AF],
                )
            # Chunks 0 and 8 read the lattice at base 0 / 32 directly;
            # chunks 1-7 are restaged to base-0 tiles off the critical path.
            rts = {0: lat[0 : _K, :], _NT - 1: lat[32 : 32 + _K, :]}
            rbase = {0: 0, _NT - 1: 32}
            for t in range(1, _NT - 1):
                rt = spool.tile([_K, _CH], _F16)
                nc.scalar.dma_start(out=rt[:, :], in_=lat[t * _K : (t + 1) * _K, :])
                rts[t] = rt[:, :]
                rbase[t] = 0

            # ---- main stream: out = x * (1 + scale) ----
            for t in range(_NT):
                nsl = slice(t * _CH, (t + 1) * _CH)
                rt = rts[t]
                wb = rbase[t]
                for ch in range(_C // _P):
                    lhsT = wt[wb : wb + _K, ch * _P : (ch + 1) * _P]
                    xt = xpool.tile([_P, _CH], _F16)
                    nc.sync.dma_start(
                        out=xt[:, :], in_=x_d[ch * _P : (ch + 1) * _P, nsl]
                    )
                    ps = pspool.tile([_P, _CH], _F32)
                    for j in range(_CH // _MM):
                        nc.tensor.matmul(
                            ps[:, j * _MM : (j + 1) * _MM],
                            lhsT,
                            rt[:, j * _MM : (j + 1) * _MM],
                            start=True,
                            stop=True,
                        )
                    ot = ypool.tile([_P, _CH], _F16)
                    nc.vector.tensor_mul(
                        out=ot[:, 0:_VS], in0=xt[:, 0:_VS], in1=ps[:, 0:_VS]
                    )
                    # GPSIMD cannot read PSUM: scalar engine stages the tail
                    # columns to SBUF (fp16), gpsimd multiplies from there.
                    cs = cpool2.tile([_P, _CH - _VS], _F16)
                    nc.scalar.copy(out=cs[:, :], in_=ps[:, _VS:])
                    nc.gpsimd.tensor_mul(
                        out=ot[:, _VS:], in0=xt[:, _VS:], in1=cs[:, :]
                    )
                    nc.scalar.dma_start(
                        out=y_d[ch * _P : (ch + 1) * _P, nsl], in_=ot[:, :]
                    )
    nc.compile()
    return nc


def _prepare_in_maps(x, quality_map, fc1_w, fc2_w, fc2_b, weight):
    x = np.asarray(x, dtype=np.float32)
    qm = np.asarray(quality_map, dtype=np.float32)
    fc1 = np.asarray(fc1_w, dtype=np.float32)
    fc2 = np.asarray(fc2_w, dtype=np.float32)
    b2 = np.asarray(fc2_b, dtype=np.float32)
    w = np.asarray(weight, dtype=np.float32)

    # Weight-only folding (host): g = fc2 @ relu(fc1); w1 = w_sum + 1.
    # w1 is placed at partition bases 0 and 32 (chunk 8's lattice rows sit
    # at base 32 and the PE requires lhsT/rhs at the same 32-aligned base).
    g = (fc2 @ np.maximum(fc1[:, 0], 0.0)).astype(np.float32)        # [K]
    w1 = (w.sum(axis=1) + 1.0).astype(np.float16)                    # [K, C]
    w4 = np.zeros((_P, _C), dtype=np.float16)
    w4[0:_K] = w1
    w4[32 : 32 + _K] = w1
    gb = np.concatenate([g / _TEMP, b2 / _TEMP]).astype(np.float32)  # [2K]
    gb_rep = np.ascontiguousarray(np.broadcast_to(gb, (_AP_, 2 * _K)))

    x16 = x.astype(np.float16)
    in_maps = []
    for core in range(_NCORES):
        b, half = divmod(core, 2)
        h0 = half * _HS
        xs = np.ascontiguousarray(x16[b, :, h0 : h0 + _HS, :]).reshape(_C, _N)
        qs = np.ascontiguousarray(qm[b, 0, h0 : h0 + _HS, :]).reshape(_AP_, _AF)
        in_maps.append({"x": xs, "qm": qs, "w": w4, "g": gb_rep})
    return in_maps


def _run(in_maps, **kwargs):
    nc = _build_nc()
    return run_bass_kernel_spmd(nc, in_maps, core_ids=list(range(_NCORES)), **kwargs)


def kernel(x, quality_map, fc1_w, fc2_w, fc2_b, weight):
    in_maps = _prepare_in_maps(x, quality_map, fc1_w, fc2_w, fc2_b, weight)
    res = _run(in_maps)
    out = np.empty((_B, _C, _H, _W), dtype=np.float32)
    for core in range(_NCORES):
        b, half = divmod(core, 2)
        h0 = half * _HS
        out[b, :, h0 : h0 + _HS, :] = (
            res.results[core]["y"].astype(np.float32).reshape(_C, _HS, _W)
        )
    return out
